# revision 21
# baseline (speedup 1.0000x reference)
"""Bass/Tile TRN2 kernel for nn_BatchGraphAttentionLayer.

Reference computation (per batch b):
    Wh  = h[b] @ W                    # [64, 256]
    s1  = Wh @ a[:256], s2 = Wh @ a[256:]
    e   = leaky_relu(s1[i] + s2[j])   # [64, 64]
    att = softmax over axis i of where(adj[i,j]>0, e, -9e15)
    out = elu(att @ Wh)               # contraction over j
Sharding: data-parallel over batch. 8 cores x 4 batches each.

Design notes (v5):
  - Single-bf16 projection (absmax-rel error well under the 2e-2 gate).
  - Scores are FREE: w_pack carries two extra bf16 columns
    w12 = [W@a1 | W@a2] (host-precomputed, exact factorization
    s1 = (h W) a1 = h (W a1)), so the projection's moving operand is
    [W | w12] (258 wide) and s1|s2 accumulate in PSUM cols 256:258.
    This deletes the whole DVE score chain and the arep constant.
  - The 4 local batches form 2 pairs of 128 rows; pair t's attention is
    a [128,128] tile with off-diagonal blocks masked to -9e15 (exactly
    0 after softmax), one [128,128]x[128,256] matmul per pair.
  - e^T[j,i] = s1[i] + s2[j] per pair via one 2-deep bf16 matmul
    (lhsT = [ones; s2], rhs = [s1; ones]; rows built by PE-transposing
    column-packed [128,2] tiles - DVE needs partition offset 0).
  - Masking in ONE op: min(leaky, mask_min), mask_min = +BIG/-9e15.
  - att and Wh cast to bf16 for the final matmuls (fp32 PE is 4x/row).
  - ALL elementwise on DVE/ACT. The Pool engine is ~26x slower than DVE
    for [128,256] tiles (measured 3.8us/op) - only tiny memsets go there.
  - All attention PE ops are emitted after the full projection (PE is
    in-order; anything earlier head-of-line-blocks the projection).
"""

import os
from contextlib import ExitStack

import ml_dtypes
import numpy as np

import concourse.bass as bass
import concourse.tile as tile
from concourse import bacc, mybir
from concourse.bass_utils import run_bass_kernel_spmd

F32 = mybir.dt.float32
BF16 = mybir.dt.bfloat16

B, N, IN, OUT = 32, 64, 16384, 256
NCORES = 8
BPC = B // NCORES            # batches per core = 4
M = BPC * N                  # local rows = 256
P = 128
WCOL = OUT + 2               # [W | w1 | w2]
NEG = -9e15
BIG = 3.0e38
ALPHA = 0.2

KSUB = IN // P               # 128 k-subtiles of 128
SLABS = [16] * 8             # 16 chunks per DMA -> all 16 queue engines
assert sum(SLABS) == KSUB
SLAB_MAX = max(SLABS)
NSLAB = len(SLABS)
# PE consumes slabs starting at T0SLAB (accumulation is commutative):
# starting a few slabs into the stream gives the PE a deep backlog so it
# runs dense and holds the 2.4 GHz DVFS clock instead of idling at 1.2.
T0SLAB = int(os.environ.get("GAT_T0SLAB", "0"))

_NC = None
LAST_EXEC_NS = None
LAST_RESULTS = None


def _build_kernel(ctx: ExitStack, tc: tile.TileContext, out, h_hi,
                  w_pack, maskmin, eye_b):
    nc = tc.nc

    consts = ctx.enter_context(tc.tile_pool(name="consts", bufs=1))
    hpool = ctx.enter_context(tc.tile_pool(name="hslab", bufs=1))
    wpool = ctx.enter_context(tc.tile_pool(name="wslab", bufs=1))
    whpool = ctx.enter_context(tc.tile_pool(name="wh", bufs=1))
    small = ctx.enter_context(tc.tile_pool(name="small", bufs=1))
    attp = ctx.enter_context(tc.tile_pool(name="att", bufs=1))
    ps_accp = ctx.enter_context(tc.tile_pool(name="psacc", bufs=1, space="PSUM"))
    ps_smallp = ctx.enter_context(tc.tile_pool(name="pssmall", bufs=1, space="PSUM"))
    ps_ep = ctx.enter_context(tc.tile_pool(name="pse", bufs=1, space="PSUM"))
    ps_op = ctx.enter_context(tc.tile_pool(name="pso", bufs=1, space="PSUM"))

    # consts via gpsimd/SWDGE (needed only in the attention tail)
    sb_eye = consts.tile([P, P], BF16)
    nc.gpsimd.dma_start(sb_eye, eye_b)
    sb_mm = consts.tile([P, 2 * P], F32)
    nc.gpsimd.dma_start(sb_mm, maskmin)

    # ---- phase 1: [Wh | s] = h @ [W | w12] accumulated in PSUM ----
    ps_wh = [ps_accp.tile([P, WCOL], F32, tag=f"ps_wh{t}", name=f"ps_wh{t}")
             for t in range(2)]
    hs, ws = [], []
    for s in range(NSLAB):
        ksl = slice(s * SLAB_MAX * P, (s + 1) * SLAB_MAX * P)
        w_ = wpool.tile([P, SLAB_MAX, WCOL], BF16, tag=f"ws{s}",
                        name=f"ws{s}")
        # W on the OPPOSITE queue from h (h is all-sync) so slab 0's
        # h and W stream in parallel and the PE starts ~5us earlier
        weng = nc.scalar if s % 2 == 0 else nc.sync
        weng.dma_start(w_[:],
                       w_pack[ksl, :].rearrange("(p c) o -> p c o", p=P))
        h_ = hpool.tile([P, SLAB_MAX, M], BF16, tag=f"hs{s}", name=f"hs{s}")
        nc.sync.dma_start(h_[:],
                          h_hi[ksl, :].rearrange("(p c) m -> p c m", p=P))
        ws.append(w_)
        hs.append(h_)
    corder = list(range(T0SLAB, NSLAB)) + list(range(T0SLAB))
    for j, s in enumerate(corder):
        last = (j >= NSLAB - 2)
        if not last:
            for c in range(SLAB_MAX):
                for t in range(2):
                    nc.tensor.matmul(ps_wh[t],
                                     lhsT=hs[s][:, c, t * P:(t + 1) * P],
                                     rhs=ws[s][:, c, :],
                                     start=(j == 0 and c == 0), stop=False,
                                     skip_group_check=True)
        else:
            # pair 0 finishes first so its attention overlaps pair 1's tail
            fin = (j == NSLAB - 1)
            for t in range(2):
                for c in range(SLAB_MAX):
                    nc.tensor.matmul(ps_wh[t],
                                     lhsT=hs[s][:, c, t * P:(t + 1) * P],
                                     rhs=ws[s][:, c, :],
                                     start=False,
                                     stop=(fin and c == SLAB_MAX - 1),
                                     skip_group_check=True)

    # ---- phase 2: per-pair attention ----
    # 2a: pack score columns (from PSUM cols 256:258) as bf16
    #     [ones | s2] / [s1 | ones] and bf16-evacuate Wh (ACT engine)
    wh_b, sca, scc = [], [], []
    for t in range(2):
        wb = whpool.tile([P, OUT], BF16, tag=f"wh_b{t}", name=f"wh_b{t}")
        nc.scalar.copy(out=wb, in_=ps_wh[t][:, :OUT])
        a_ = small.tile([P, 2], BF16, tag=f"sc_a{t}")   # [ones | s2]
        c_ = small.tile([P, 2], BF16, tag=f"sc_c{t}")   # [s1 | ones]
        nc.gpsimd.memset(a_[:, 0:1], 1.0)
        nc.gpsimd.memset(c_[:, 1:2], 1.0)
        nc.vector.tensor_copy(out=c_[:, 0:1], in_=ps_wh[t][:, OUT:OUT + 1])
        nc.vector.tensor_copy(out=a_[:, 1:2], in_=ps_wh[t][:, OUT + 1:OUT + 2])
        wh_b.append(wb)
        sca.append(a_)
        scc.append(c_)

    # 2b: transposes + e-build matmuls for both pairs (PE, in order)
    lhs_t, rhs_t, ps_e = [], [], []
    for t in range(2):
        pl = ps_smallp.tile([2, P], BF16, tag="ps_l", name=f"ps_l{t}")
        nc.tensor.transpose(pl, sca[t], sb_eye)
        pr = ps_smallp.tile([2, P], BF16, tag="ps_r", name=f"ps_r{t}")
        nc.tensor.transpose(pr, scc[t], sb_eye)
        lt = small.tile([2, P], BF16, tag=f"eb_l{t}")
        rt = small.tile([2, P], BF16, tag=f"eb_r{t}")
        nc.scalar.copy(out=lt, in_=pl)
        nc.scalar.copy(out=rt, in_=pr)
        lhs_t.append(lt)
        rhs_t.append(rt)
    pe2 = ps_ep.tile([P, 2 * P], F32, tag="ps_e2", name="ps_e2")
    for t in range(2):
        nc.tensor.matmul(pe2[:, t * P:(t + 1) * P], lhsT=lhs_t[t],
                         rhs=rhs_t[t], start=True, stop=True,
                         skip_group_check=True)
        ps_e.append(pe2[:, t * P:(t + 1) * P])

    # 2c: leaky = max(v, 0.2v) on DVE over BOTH pairs at once [128,256]
    # (ACT Lrelu's alpha is NOT honored - measured 8e-2 error), mask via
    # one DVE min (mask doubled host-side), exp per pair (per-pair row
    # sums), then softmax scale as an ACT per-partition-scale copy.
    vb = attp.tile([P, 2 * P], F32, tag="vb")
    nc.vector.tensor_scalar(vb, pe2, ALPHA, None, mybir.AluOpType.mult)
    lk2 = attp.tile([P, 2 * P], F32, tag="lk2")
    nc.vector.tensor_tensor(lk2, pe2, vb, mybir.AluOpType.max)
    lkm2 = attp.tile([P, 2 * P], F32, tag="lkm2")
    nc.vector.tensor_tensor(lkm2, lk2, sb_mm, mybir.AluOpType.min)
    att_b = []
    for t in range(2):
        p_ = attp.tile([P, P], F32, tag=f"pexp{t}")
        r_ = small.tile([P, 1], F32, tag=f"rsum{t}")
        nc.scalar.activation(p_, lkm2[:, t * P:(t + 1) * P],
                             mybir.ActivationFunctionType.Exp,
                             scale=1.0, accum_out=r_)
        rinv = small.tile([P, 1], F32, tag=f"rinv{t}")
        nc.vector.reciprocal(rinv, r_)
        ab = attp.tile([P, P], BF16, tag=f"att_b{t}")
        nc.scalar.activation(ab, p_, mybir.ActivationFunctionType.Copy,
                             scale=rinv)
        att_b.append(ab)
    ps_o = []
    for t in range(2):
        po = ps_op.tile([P, OUT], F32, tag=f"ps_o{t}", name=f"ps_o{t}")
        nc.tensor.matmul(po, lhsT=att_b[t], rhs=wh_b[t], start=True,
                         stop=True, skip_group_check=True)
        ps_o.append(po)
    for t in range(2):
        # elu(x) = relu(x) - relu(1 - exp(x))   (x <= ~15 so exp is finite;
        # min(exp,1)-1 == -relu(1-exp)). Both relus + exp run on ACT
        # straight from PSUM/SBUF; DVE does one subtract per pair.
        ex = attp.tile([P, OUT], F32, tag=f"ex{t}")
        nc.scalar.activation(ex, ps_o[t], mybir.ActivationFunctionType.Exp)
        r1e = attp.tile([P, OUT], F32, tag=f"r1e{t}")
        nc.scalar.activation(r1e, ex, mybir.ActivationFunctionType.Relu,
                             scale=-1.0, bias=1.0)
        rm = attp.tile([P, OUT], F32, tag=f"rm{t}")
        nc.scalar.activation(rm, ps_o[t], mybir.ActivationFunctionType.Relu)
        ot = attp.tile([P, OUT], F32, tag=f"ot{t}")
        nc.vector.tensor_tensor(ot, rm, r1e, mybir.AluOpType.subtract)
        oeng = nc.sync if t == 0 else nc.scalar
        oeng.dma_start(out[t * P:(t + 1) * P, :], ot)


def _get_nc():
    global _NC
    if _NC is not None:
        return _NC
    nc = bacc.Bacc("TRN2", target_bir_lowering=False, debug=False,
                   num_devices=NCORES, disable_frame_to_traceback=True,
                   enable_partition_id=False)
    h_hi = nc.dram_tensor("h_hi", [IN, M], BF16, kind="ExternalInput").ap()
    w_pack = nc.dram_tensor("w_pack", [IN, WCOL], BF16,
                            kind="ExternalInput").ap()
    maskmin = nc.dram_tensor("maskmin", [P, 2 * P], F32,
                             kind="ExternalInput").ap()
    eye_b = nc.dram_tensor("eye_b", [P, P], BF16, kind="ExternalInput").ap()
    out = nc.dram_tensor("out", [M, OUT], F32, kind="ExternalOutput").ap()
    with tile.TileContext(nc) as tc:
        with ExitStack() as ctx:
            _build_kernel(ctx, tc, out, h_hi, w_pack, maskmin, eye_b)
    nc.compile()
    _NC = nc
    return nc


def _mask_min(adj: np.ndarray):
    adjb = (np.asarray(adj) > 0)                 # [i, j]
    mm = np.full((P, P), np.float32(NEG), np.float32)
    sel = adjb.T                                 # [j, i]
    mm[:N, :N][sel] = BIG
    mm[N:, N:][sel] = BIG
    return np.ascontiguousarray(np.concatenate([mm, mm], axis=1))


def kernel(h: np.ndarray, adj: np.ndarray, W: np.ndarray, a: np.ndarray
           ) -> np.ndarray:
    global LAST_EXEC_NS, LAST_RESULTS
    h = np.asarray(h, dtype=np.float32)
    W = np.asarray(W, dtype=np.float32)
    a = np.ascontiguousarray(np.asarray(a, dtype=np.float32)).reshape(2 * OUT)
    assert h.shape == (B, N, IN) and W.shape == (IN, OUT)

    nc = _get_nc()
    mm = _mask_min(adj)
    eye_b = np.eye(P, dtype=ml_dtypes.bfloat16)
    # w12 = [W@a1 | W@a2]: exact factorization s = h @ (W @ a_q)
    w12 = (W.astype(np.float64) @ a.astype(np.float64).reshape(2, OUT).T)
    w_pack = np.concatenate(
        [W, w12.astype(np.float32)], axis=1).astype(ml_dtypes.bfloat16)
    w_pack = np.ascontiguousarray(w_pack)

    in_maps = []
    for c in range(NCORES):
        hT = h[c * BPC:(c + 1) * BPC].reshape(M, IN).T
        imap = {"h_hi": np.ascontiguousarray(hT).astype(ml_dtypes.bfloat16),
                "w_pack": w_pack, "maskmin": mm, "eye_b": eye_b}
        in_maps.append(imap)

    trace = os.environ.get("GAT_TRACE", "0") == "1"
    res = run_bass_kernel_spmd(nc, in_maps, list(range(NCORES)), trace=trace)
    LAST_EXEC_NS = res.exec_time_ns
    LAST_RESULTS = res

    out = np.empty((B, N, OUT), np.float32)
    for c in range(NCORES):
        out[c * BPC:(c + 1) * BPC] = res.results[c]["out"].reshape(BPC, N, OUT)
    return out
